# revision 5
# baseline (speedup 1.0000x reference)
"""Nearest-color-distance loss on 8 TRN2 NeuronCores.

loss = mean_i min_j ||x_i - p_j||_2,  x: (131072, 3), p: (128, 3).

Host-side candidate pruning turns the dense 16384x128 distance problem
into ~2300 matmul columns per core:
 - colors are sorted spatially (8 slabs by x0 -> cores; per core 16
   strips by x1 x 8 cells by x2) into 128 chunks of 128 colors each,
 - for each chunk the host computes the EXACT set of palette entries
   that can be the nearest neighbour of any point in the chunk's
   bounding box (min-dist(box, p_j) <= min_k max-dist(box, p_k)),
 - chunks get 16-wide candidate slots (mean |S| ~ 8, max 27 -> 1-2
   slots), 16 slots per matmul: 9 matmuls of [112,128] x [112,256].
Payloads are fp16 with exact quantized geometry: rows per slot are
[x1,x2,x3,xn_hi,xn_lo,1,1] against [-2p1,-2p2,-2p3,1,1,pn_hi,pn_lo],
so PSUM fp32 holds ||x16 - p16||^2 to ~1e-7 and the only error is the
fp16 quantization of the points themselves (~2.4e-4 per coordinate).
DVE min-reduces each PSUM bank ([128,32,16] -> [128,32]); one fp32
output DMA per core; the host combines overflow slots, does
sqrt/mean in f64.
"""

import sys

sys.path.insert(0, "/opt/trn_rl_repo")

import numpy as np

import concourse.bass as bass
import concourse.tile as tile
from concourse import bacc, mybir
from concourse.alu_op_type import AluOpType
from concourse.bass_utils import run_bass_kernel_spmd

N_CORES = 8
N = 131072
NPC = N // N_CORES  # 16384 colors per core
NP = 128  # palette size
ROWS = 5  # rows per slot
G = 16  # slots per matmul
K = ROWS * G  # 80 contraction rows
MSL = 16  # candidates per slot
NMM = 9  # matmuls per core
NSLOT = NMM * G  # 144 slots
F32 = mybir.dt.float32
F16 = mybir.dt.float16
AX = mybir.AxisListType


def build_nc():
    nc = bacc.Bacc(
        "TRN2",
        target_bir_lowering=False,
        debug=False,
        enable_asserts=False,
        num_devices=N_CORES,
    )
    # g0's stationary+moving first so the PE can start ASAP
    bun0_d = nc.dram_tensor("bun0", [K, 384], F16, kind="ExternalInput").ap()
    buna_d = nc.dram_tensor("buna", [K, 768], F16, kind="ExternalInput").ap()
    bunb_d = nc.dram_tensor("bunb", [K, 768], F16, kind="ExternalInput").ap()
    bunc_d = nc.dram_tensor("bunc", [K, 768], F16, kind="ExternalInput").ap()
    bund_d = nc.dram_tensor("bund", [K, 768], F16, kind="ExternalInput").ap()
    out1_d = nc.dram_tensor("mind2a", [128, 96], F32, kind="ExternalOutput").ap()
    out2_d = nc.dram_tensor("mind2b", [128, 48], F32, kind="ExternalOutput").ap()

    with tile.TileContext(nc) as tc:
        with (
            tc.tile_pool(name="sb", bufs=1) as sb,
            tc.tile_pool(name="pp", bufs=5, space=bass.MemorySpace.PSUM) as pp,
        ):
            buf = sb.tile([K, 3456], F16)
            outs = sb.tile([128, NSLOT], F32)
            nc.sync.dma_start(buf[:, 0:384], bun0_d)
            nc.scalar.dma_start(buf[:, 384:1152], buna_d)
            nc.gpsimd.dma_start(buf[:, 1152:1920], bunb_d)
            nc.scalar.dma_start(buf[:, 1920:2688], bunc_d)
            nc.sync.dma_start(buf[:, 2688:3456], bund_d)

            def xt_g(g):
                return buf[:, 384 * g : 384 * g + 128]

            def pm_g(g):
                return buf[:, 384 * g + 128 : 384 * (g + 1)]

            for g in range(NMM):
                if g % 2 == 0:
                    ps = pp.tile([128, 512], F32, tag="ps")
                h = g % 2
                nc.tensor.matmul(
                    ps[:, 256 * h : 256 * (h + 1)],
                    xt_g(g),
                    pm_g(g),
                    start=True,
                    stop=True,
                )
                nc.vector.tensor_reduce(
                    outs[:, 16 * g : 16 * (g + 1)],
                    ps[:, 256 * h : 256 * (h + 1)].rearrange(
                        "p (s k) -> p s k", k=MSL
                    ),
                    axis=AX.X,
                    op=AluOpType.min,
                )
                if g == 5:
                    nc.scalar.dma_start(out1_d[:], outs[:, 0:96])
            nc.sync.dma_start(out2_d[:], outs[:, 96:144])


    nc.compile()
    return nc


def prep_inputs(output_colors, target_palette):
    pal = np.asarray(target_palette, dtype=np.float32)
    mu = pal.mean(axis=0)
    p16 = (pal - mu).astype(np.float16)
    p64 = p16.astype(np.float64)  # exact values of the quantized palette
    pn64 = (p64 * p64).sum(axis=1)
    pnh = pn64.astype(np.float16)
    pnl = (pn64 - pnh.astype(np.float64)).astype(np.float16)
    # per-candidate 5-row payload [5, 128]
    prow = np.zeros((ROWS, NP), dtype=np.float16)
    prow[0:3] = (-2.0 * p64).astype(np.float16).T  # exact: 2*fp16 is exact
    prow[3] = pnh
    prow[4] = pnl

    x16 = (np.asarray(output_colors, dtype=np.float32) - mu).astype(np.float16)
    x64 = x16.astype(np.float64)

    order = np.argsort(x64[:, 0], kind="stable")
    x64 = x64[order]

    in_maps = []
    metas = []
    for c in range(N_CORES):
        xs = x64[c * NPC : (c + 1) * NPC]
        o1 = np.argsort(xs[:, 1], kind="stable")
        xs = xs[o1]
        parts = []
        for s in range(16):
            strip = xs[s * 1024 : (s + 1) * 1024]
            o2 = np.argsort(strip[:, 2], kind="stable")
            parts.append(strip[o2])
        xs = np.concatenate(parts, axis=0)
        ch = xs.reshape(128, 128, 3)  # [chunk, color, coord]

        # exact candidate sets per chunk
        lo = ch.min(axis=1)[:, None, :]  # [128,1,3]
        hi = ch.max(axis=1)[:, None, :]
        d_out = np.maximum(np.maximum(lo - p64, p64 - hi), 0.0)
        mind = np.sqrt((d_out**2).sum(-1))  # [chunk, pal]
        far = np.maximum(np.abs(p64 - lo), np.abs(p64 - hi))
        maxd = np.sqrt((far**2).sum(-1))
        rB = maxd.min(axis=1) + 1e-9  # [chunk]
        keep = mind <= rB[:, None]

        # x-side rows per chunk: [5, 128]; xn added on the host
        xn64 = (ch * ch).sum(-1)  # [chunk, color]
        xrows = np.zeros((128, ROWS, 128), dtype=np.float16)
        xrows[:, 0:3] = ch.astype(np.float16).transpose(0, 2, 1)
        xrows[:, 3:5] = 1.0

        xp = np.zeros((K, NMM * 384), dtype=np.float16)
        slot_chunk = np.full(NSLOT, -1, dtype=np.int32)
        s = 0
        for cidx in range(128):
            cands = np.flatnonzero(keep[cidx])
            for st in range(0, len(cands), MSL):
                sub = cands[st : st + MSL]
                if len(sub) < MSL:
                    sub = np.concatenate(
                        [sub, np.full(MSL - len(sub), cands[0], dtype=sub.dtype)]
                    )
                g, pos = divmod(s, G)
                xp[ROWS * pos : ROWS * (pos + 1), 384 * g : 384 * g + 128] = (
                    xrows[cidx]
                )
                pm0 = 384 * g + 128
                xp[
                    ROWS * pos : ROWS * (pos + 1),
                    pm0 + MSL * pos : pm0 + MSL * (pos + 1),
                ] = prow[:, sub]
                slot_chunk[s] = cidx
                s += 1
        assert s <= NSLOT, f"core {c}: {s} slots > {NSLOT}"

        in_maps.append(
            {
                "bun0": np.ascontiguousarray(xp[:, 0:384]),
                "buna": np.ascontiguousarray(xp[:, 384:1152]),
                "bunb": np.ascontiguousarray(xp[:, 1152:1920]),
                "bunc": np.ascontiguousarray(xp[:, 1920:2688]),
                "bund": np.ascontiguousarray(xp[:, 2688:3456]),
            }
        )
        metas.append((slot_chunk, xn64))
    return in_maps, metas


_NC_CACHE = {}


def get_nc():
    if "nc" not in _NC_CACHE:
        _NC_CACHE["nc"] = build_nc()
    return _NC_CACHE["nc"]


def kernel(output_colors=None, target_palette=None, _trace=False, **_):
    nc = get_nc()
    in_maps, metas = prep_inputs(output_colors, target_palette)
    res = run_bass_kernel_spmd(
        nc, in_maps, core_ids=list(range(N_CORES)), trace=_trace
    )
    total = np.float64(0.0)
    for r, (slot_chunk, xn64) in zip(res.results, metas):
        md = np.concatenate([r["mind2a"], r["mind2b"]], axis=1).astype(
            np.float64
        )  # [128 colors, 144 slots]
        mins = np.full((128, 128), np.inf)  # [chunk, color]
        for s in range(NSLOT):
            c = slot_chunk[s]
            if c >= 0:
                mins[c] = np.minimum(mins[c], md[:, s])
        d2 = mins + xn64  # [chunk, color]
        total += np.sqrt(np.maximum(d2, 0.0)).sum()
    out = np.array(total / N, dtype=np.float32)
    if _trace:
        kernel._last_results = res
    return out


if __name__ == "__main__":
    rng = np.random.default_rng(0)
    oc = rng.random((N, 3), dtype=np.float32)
    tp = rng.random((NP, 3), dtype=np.float32)
    got = kernel(output_colors=oc, target_palette=tp)
    d = oc[:, None, :] - tp[None, :, :]
    want = np.sqrt((d * d).sum(-1)).min(1).mean(dtype=np.float64)
    print("got", got, "want", want, "rel", abs(got - want) / abs(want))


# revision 6
# speedup vs baseline: 1.1477x; 1.1477x over previous
"""Nearest-color-distance loss on 8 TRN2 NeuronCores.

loss = mean_i min_j ||x_i - p_j||_2,  x: (131072, 3), p: (128, 3).

Host-side candidate pruning turns the dense 16384x128 distance problem
into ~2300 matmul columns per core:
 - colors are sorted spatially (8 slabs by x0 -> cores; per core 16
   strips by x1 x 8 cells by x2) into 128 chunks of 128 colors each,
 - for each chunk the host computes the EXACT set of palette entries
   that can be the nearest neighbour of any point in the chunk's
   bounding box (min-dist(box, p_j) <= min_k max-dist(box, p_k)),
 - chunks get 16-wide candidate slots (mean |S| ~ 8, max 27 -> 1-2
   slots), 16 slots per matmul: 9 matmuls of [112,128] x [112,256].
Payloads are fp16 with exact quantized geometry: rows per slot are
[x1,x2,x3,xn_hi,xn_lo,1,1] against [-2p1,-2p2,-2p3,1,1,pn_hi,pn_lo],
so PSUM fp32 holds ||x16 - p16||^2 to ~1e-7 and the only error is the
fp16 quantization of the points themselves (~2.4e-4 per coordinate).
DVE min-reduces each PSUM bank ([128,32,16] -> [128,32]); one fp32
output DMA per core; the host combines overflow slots, does
sqrt/mean in f64.
"""

import sys

sys.path.insert(0, "/opt/trn_rl_repo")

import numpy as np

import concourse.bass as bass
import concourse.tile as tile
from concourse import bacc, mybir
from concourse.alu_op_type import AluOpType
from concourse.bass_utils import run_bass_kernel_spmd

N_CORES = 8
N = 131072
NPC = N // N_CORES  # 16384 colors per core
NP = 128  # palette size
ROWS = 5  # rows per slot
G = 16  # slots per matmul
K = ROWS * G  # 80 contraction rows
MSL = 16  # candidates per slot
NMM = 9  # matmuls per core
NSLOT = NMM * G  # 144 slots
F32 = mybir.dt.float32
F16 = mybir.dt.float16
AX = mybir.AxisListType


def build_nc():
    nc = bacc.Bacc(
        "TRN2",
        target_bir_lowering=False,
        debug=False,
        enable_asserts=False,
        num_devices=N_CORES,
    )
    # g0's stationary+moving first so the PE can start ASAP
    bun0_d = nc.dram_tensor("bun0", [K, 768], F16, kind="ExternalInput").ap()
    buna_d = nc.dram_tensor("buna", [K, 1152], F16, kind="ExternalInput").ap()
    bunb_d = nc.dram_tensor("bunb", [K, 1152], F16, kind="ExternalInput").ap()
    bunc_d = nc.dram_tensor("bunc", [K, 384], F16, kind="ExternalInput").ap()
    out1_d = nc.dram_tensor("mind2a", [128, 96], F32, kind="ExternalOutput").ap()
    out2_d = nc.dram_tensor("mind2b", [128, 48], F32, kind="ExternalOutput").ap()

    with tile.TileContext(nc) as tc:
        with (
            tc.tile_pool(name="sb", bufs=1) as sb,
            tc.tile_pool(name="pp", bufs=5, space=bass.MemorySpace.PSUM) as pp,
        ):
            buf = sb.tile([K, 3456], F16)
            outs = sb.tile([128, NSLOT], F32)
            nc.sync.dma_start(buf[:, 0:768], bun0_d)
            nc.scalar.dma_start(buf[:, 768:1920], buna_d)
            nc.gpsimd.dma_start(buf[:, 1920:3072], bunb_d)
            nc.sync.dma_start(buf[:, 3072:3456], bunc_d)

            def xt_g(g):
                return buf[:, 384 * g : 384 * g + 128]

            def pm_g(g):
                return buf[:, 384 * g + 128 : 384 * (g + 1)]

            for b in range(4):
                ps = pp.tile([128, 512], F32, tag="ps")
                for h in range(2):
                    g = 2 * b + h
                    nc.tensor.matmul(
                        ps[:, 256 * h : 256 * (h + 1)],
                        xt_g(g),
                        pm_g(g),
                        start=True,
                        stop=True,
                    )
                nc.vector.tensor_reduce(
                    outs[:, 32 * b : 32 * (b + 1)],
                    ps[:].rearrange("p (s k) -> p s k", k=MSL),
                    axis=AX.X,
                    op=AluOpType.min,
                )
                if b == 2:
                    nc.scalar.dma_start(out1_d[:], outs[:, 0:96])
            ps = pp.tile([128, 512], F32, tag="ps")
            nc.tensor.matmul(
                ps[:, 0:256], xt_g(8), pm_g(8), start=True, stop=True
            )
            nc.vector.tensor_reduce(
                outs[:, 128:144],
                ps[:, 0:256].rearrange("p (s k) -> p s k", k=MSL),
                axis=AX.X,
                op=AluOpType.min,
            )
            nc.sync.dma_start(out2_d[:], outs[:, 96:144])


    nc.compile()
    return nc


def prep_inputs(output_colors, target_palette):
    pal = np.asarray(target_palette, dtype=np.float32)
    mu = pal.mean(axis=0)
    p16 = (pal - mu).astype(np.float16)
    p64 = p16.astype(np.float64)  # exact values of the quantized palette
    pn64 = (p64 * p64).sum(axis=1)
    pnh = pn64.astype(np.float16)
    pnl = (pn64 - pnh.astype(np.float64)).astype(np.float16)
    # per-candidate 5-row payload [5, 128]
    prow = np.zeros((ROWS, NP), dtype=np.float16)
    prow[0:3] = (-2.0 * p64).astype(np.float16).T  # exact: 2*fp16 is exact
    prow[3] = pnh
    prow[4] = pnl

    x16 = (np.asarray(output_colors, dtype=np.float32) - mu).astype(np.float16)
    x64 = x16.astype(np.float64)

    order = np.argsort(x64[:, 0], kind="stable")
    x64 = x64[order]

    in_maps = []
    metas = []
    for c in range(N_CORES):
        xs = x64[c * NPC : (c + 1) * NPC]
        o1 = np.argsort(xs[:, 1], kind="stable")
        xs = xs[o1]
        parts = []
        for s in range(16):
            strip = xs[s * 1024 : (s + 1) * 1024]
            o2 = np.argsort(strip[:, 2], kind="stable")
            parts.append(strip[o2])
        xs = np.concatenate(parts, axis=0)
        ch = xs.reshape(128, 128, 3)  # [chunk, color, coord]

        # exact candidate sets per chunk
        lo = ch.min(axis=1)[:, None, :]  # [128,1,3]
        hi = ch.max(axis=1)[:, None, :]
        d_out = np.maximum(np.maximum(lo - p64, p64 - hi), 0.0)
        mind = np.sqrt((d_out**2).sum(-1))  # [chunk, pal]
        far = np.maximum(np.abs(p64 - lo), np.abs(p64 - hi))
        maxd = np.sqrt((far**2).sum(-1))
        rB = maxd.min(axis=1) + 1e-9  # [chunk]
        keep = mind <= rB[:, None]

        # x-side rows per chunk: [5, 128]; xn added on the host
        xn64 = (ch * ch).sum(-1)  # [chunk, color]
        xrows = np.zeros((128, ROWS, 128), dtype=np.float16)
        xrows[:, 0:3] = ch.astype(np.float16).transpose(0, 2, 1)
        xrows[:, 3:5] = 1.0

        xp = np.zeros((K, NMM * 384), dtype=np.float16)
        slot_chunk = np.full(NSLOT, -1, dtype=np.int32)
        s = 0
        for cidx in range(128):
            cands = np.flatnonzero(keep[cidx])
            for st in range(0, len(cands), MSL):
                sub = cands[st : st + MSL]
                if len(sub) < MSL:
                    sub = np.concatenate(
                        [sub, np.full(MSL - len(sub), cands[0], dtype=sub.dtype)]
                    )
                g, pos = divmod(s, G)
                xp[ROWS * pos : ROWS * (pos + 1), 384 * g : 384 * g + 128] = (
                    xrows[cidx]
                )
                pm0 = 384 * g + 128
                xp[
                    ROWS * pos : ROWS * (pos + 1),
                    pm0 + MSL * pos : pm0 + MSL * (pos + 1),
                ] = prow[:, sub]
                slot_chunk[s] = cidx
                s += 1
        assert s <= NSLOT, f"core {c}: {s} slots > {NSLOT}"

        in_maps.append(
            {
                "bun0": np.ascontiguousarray(xp[:, 0:768]),
                "buna": np.ascontiguousarray(xp[:, 768:1920]),
                "bunb": np.ascontiguousarray(xp[:, 1920:3072]),
                "bunc": np.ascontiguousarray(xp[:, 3072:3456]),
            }
        )
        metas.append((slot_chunk, xn64))
    return in_maps, metas


_NC_CACHE = {}


def get_nc():
    if "nc" not in _NC_CACHE:
        _NC_CACHE["nc"] = build_nc()
    return _NC_CACHE["nc"]


def kernel(output_colors=None, target_palette=None, _trace=False, **_):
    nc = get_nc()
    in_maps, metas = prep_inputs(output_colors, target_palette)
    res = run_bass_kernel_spmd(
        nc, in_maps, core_ids=list(range(N_CORES)), trace=_trace
    )
    total = np.float64(0.0)
    for r, (slot_chunk, xn64) in zip(res.results, metas):
        md = np.concatenate([r["mind2a"], r["mind2b"]], axis=1).astype(
            np.float64
        )  # [128 colors, 144 slots]
        mins = np.full((128, 128), np.inf)  # [chunk, color]
        for s in range(NSLOT):
            c = slot_chunk[s]
            if c >= 0:
                mins[c] = np.minimum(mins[c], md[:, s])
        d2 = mins + xn64  # [chunk, color]
        total += np.sqrt(np.maximum(d2, 0.0)).sum()
    out = np.array(total / N, dtype=np.float32)
    if _trace:
        kernel._last_results = res
    return out


if __name__ == "__main__":
    rng = np.random.default_rng(0)
    oc = rng.random((N, 3), dtype=np.float32)
    tp = rng.random((NP, 3), dtype=np.float32)
    got = kernel(output_colors=oc, target_palette=tp)
    d = oc[:, None, :] - tp[None, :, :]
    want = np.sqrt((d * d).sum(-1)).min(1).mean(dtype=np.float64)
    print("got", got, "want", want, "rel", abs(got - want) / abs(want))


# revision 7
# speedup vs baseline: 1.1867x; 1.0340x over previous
"""Nearest-color-distance loss on 8 TRN2 NeuronCores.

loss = mean_i min_j ||x_i - p_j||_2,  x: (131072, 3), p: (128, 3).

Host-side candidate pruning turns the dense 16384x128 distance problem
into ~1900 matmul columns per core:
 - colors are sorted spatially (8 slabs by x0 -> cores; per core 16
   strips by x1 x 8 cells by x2) into 128 chunks of 128 colors each,
 - for each chunk the host computes the EXACT set of palette entries
   that can be the nearest neighbour of any point in the chunk's
   bounding box (min-dist(box, p_j) <= min_k max-dist(box, p_k)),
 - chunks get 12-wide candidate slots (mean |S| ~ 8, max 27 -> 1-3
   slots), 16 slots per matmul: 10 matmuls of [80,128] x [80,192].
Payloads are fp16 with exact quantized geometry: rows per slot are
[x1,x2,x3,1,1] against [-2p1,-2p2,-2p3,pn_hi,pn_lo], so PSUM fp32
holds (pn - 2 x.p) for the fp16-quantized points to ~1e-7; the host
adds ||x||^2 in f64. The only real error is the fp16 quantization of
the points (~2.4e-4 per coordinate).
DVE min-reduces each PSUM bank ([128,32,12] -> [128,32]); two output
DMAs per core (3 banks overlapped, 2 at the end); the host combines
overflow slots, adds xn, does sqrt/mean in f64.
"""

import sys

sys.path.insert(0, "/opt/trn_rl_repo")

import numpy as np

import concourse.bass as bass
import concourse.tile as tile
from concourse import bacc, mybir
from concourse.alu_op_type import AluOpType
from concourse.bass_utils import run_bass_kernel_spmd

N_CORES = 8
N = 131072
NPC = N // N_CORES  # 16384 colors per core
NP = 128  # palette size
ROWS = 5  # rows per slot
G = 16  # slots per matmul
K = ROWS * G  # 80 contraction rows
MSL = 12  # candidates per slot
NMM = 10  # matmuls per core
NSLOT = NMM * G  # 160 slots
GW = 128 + MSL * G  # 320 cols per group bundle (xt 128 + pm 192)
F32 = mybir.dt.float32
F16 = mybir.dt.float16
AX = mybir.AxisListType


def build_nc():
    nc = bacc.Bacc(
        "TRN2",
        target_bir_lowering=False,
        debug=False,
        enable_asserts=False,
        num_devices=N_CORES,
    )
    # groups bundled (xt | pm) in need order; g0-1 first so PE starts ASAP
    bun0_d = nc.dram_tensor("bun0", [K, 2 * GW], F16, kind="ExternalInput").ap()
    buna_d = nc.dram_tensor("buna", [K, 3 * GW], F16, kind="ExternalInput").ap()
    bunb_d = nc.dram_tensor("bunb", [K, 3 * GW], F16, kind="ExternalInput").ap()
    bunc_d = nc.dram_tensor("bunc", [K, 2 * GW], F16, kind="ExternalInput").ap()
    out1_d = nc.dram_tensor("mind2a", [128, 96], F32, kind="ExternalOutput").ap()
    out2_d = nc.dram_tensor("mind2b", [128, 64], F32, kind="ExternalOutput").ap()

    with tile.TileContext(nc) as tc:
        with (
            tc.tile_pool(name="sb", bufs=1) as sb,
            tc.tile_pool(name="pp", bufs=5, space=bass.MemorySpace.PSUM) as pp,
        ):
            buf = sb.tile([K, NMM * GW], F16)
            outs = sb.tile([128, NSLOT], F32)
            nc.sync.dma_start(buf[:, 0 : 2 * GW], bun0_d)
            nc.scalar.dma_start(buf[:, 2 * GW : 5 * GW], buna_d)
            nc.gpsimd.dma_start(buf[:, 5 * GW : 8 * GW], bunb_d)
            nc.sync.dma_start(buf[:, 8 * GW : 10 * GW], bunc_d)

            def xt_g(g):
                return buf[:, GW * g : GW * g + 128]

            def pm_g(g):
                return buf[:, GW * g + 128 : GW * (g + 1)]

            for b in range(5):
                ps = pp.tile([128, 2 * MSL * G], F32, tag="ps")
                for h in range(2):
                    g = 2 * b + h
                    nc.tensor.matmul(
                        ps[:, MSL * G * h : MSL * G * (h + 1)],
                        xt_g(g),
                        pm_g(g),
                        start=True,
                        stop=True,
                    )
                nc.vector.tensor_reduce(
                    outs[:, 32 * b : 32 * (b + 1)],
                    ps[:].rearrange("p (s k) -> p s k", k=MSL),
                    axis=AX.X,
                    op=AluOpType.min,
                )
                if b == 2:
                    nc.scalar.dma_start(out1_d[:], outs[:, 0:96])
            nc.sync.dma_start(out2_d[:], outs[:, 96:160])

    nc.compile()
    return nc


def prep_inputs(output_colors, target_palette):
    pal = np.asarray(target_palette, dtype=np.float32)
    mu = pal.mean(axis=0)
    p16 = (pal - mu).astype(np.float16)
    p64 = p16.astype(np.float64)  # exact values of the quantized palette
    pn64 = (p64 * p64).sum(axis=1)
    pnh = pn64.astype(np.float16)
    pnl = (pn64 - pnh.astype(np.float64)).astype(np.float16)
    # per-candidate 5-row payload [5, 128]
    prow = np.zeros((ROWS, NP), dtype=np.float16)
    prow[0:3] = (-2.0 * p64).astype(np.float16).T  # exact: 2*fp16 is exact
    prow[3] = pnh
    prow[4] = pnl

    x16 = (np.asarray(output_colors, dtype=np.float32) - mu).astype(np.float16)
    x64 = x16.astype(np.float64)

    order = np.argsort(x64[:, 0], kind="stable")
    x64 = x64[order]

    in_maps = []
    metas = []
    for c in range(N_CORES):
        xs = x64[c * NPC : (c + 1) * NPC]
        o1 = np.argsort(xs[:, 1], kind="stable")
        xs = xs[o1]
        parts = []
        for s in range(16):
            strip = xs[s * 1024 : (s + 1) * 1024]
            o2 = np.argsort(strip[:, 2], kind="stable")
            parts.append(strip[o2])
        xs = np.concatenate(parts, axis=0)
        ch = xs.reshape(128, 128, 3)  # [chunk, color, coord]

        # exact candidate sets per chunk
        lo = ch.min(axis=1)[:, None, :]  # [128,1,3]
        hi = ch.max(axis=1)[:, None, :]
        d_out = np.maximum(np.maximum(lo - p64, p64 - hi), 0.0)
        mind = np.sqrt((d_out**2).sum(-1))  # [chunk, pal]
        far = np.maximum(np.abs(p64 - lo), np.abs(p64 - hi))
        maxd = np.sqrt((far**2).sum(-1))
        rB = maxd.min(axis=1) + 1e-9  # [chunk]
        keep = mind <= rB[:, None]

        # x-side rows per chunk: [5, 128]; xn added on the host
        xn64 = (ch * ch).sum(-1)  # [chunk, color]
        xrows = np.zeros((128, ROWS, 128), dtype=np.float16)
        xrows[:, 0:3] = ch.astype(np.float16).transpose(0, 2, 1)
        xrows[:, 3:5] = 1.0

        xp = np.zeros((K, NMM * GW), dtype=np.float16)
        slot_chunk = np.full(NSLOT, -1, dtype=np.int32)
        s = 0
        for cidx in range(128):
            cands = np.flatnonzero(keep[cidx])
            for st in range(0, len(cands), MSL):
                sub = cands[st : st + MSL]
                if len(sub) < MSL:
                    sub = np.concatenate(
                        [sub, np.full(MSL - len(sub), cands[0], dtype=sub.dtype)]
                    )
                g, pos = divmod(s, G)
                xp[ROWS * pos : ROWS * (pos + 1), GW * g : GW * g + 128] = (
                    xrows[cidx]
                )
                pm0 = GW * g + 128
                xp[
                    ROWS * pos : ROWS * (pos + 1),
                    pm0 + MSL * pos : pm0 + MSL * (pos + 1),
                ] = prow[:, sub]
                slot_chunk[s] = cidx
                s += 1
        assert s <= NSLOT, f"core {c}: {s} slots > {NSLOT}"

        in_maps.append(
            {
                "bun0": np.ascontiguousarray(xp[:, 0 : 2 * GW]),
                "buna": np.ascontiguousarray(xp[:, 2 * GW : 5 * GW]),
                "bunb": np.ascontiguousarray(xp[:, 5 * GW : 8 * GW]),
                "bunc": np.ascontiguousarray(xp[:, 8 * GW : 10 * GW]),
            }
        )
        metas.append((slot_chunk, xn64))
    return in_maps, metas


_NC_CACHE = {}


def get_nc():
    if "nc" not in _NC_CACHE:
        _NC_CACHE["nc"] = build_nc()
    return _NC_CACHE["nc"]


def kernel(output_colors=None, target_palette=None, _trace=False, **_):
    nc = get_nc()
    in_maps, metas = prep_inputs(output_colors, target_palette)
    res = run_bass_kernel_spmd(
        nc, in_maps, core_ids=list(range(N_CORES)), trace=_trace
    )
    total = np.float64(0.0)
    for r, (slot_chunk, xn64) in zip(res.results, metas):
        md = np.concatenate([r["mind2a"], r["mind2b"]], axis=1).astype(
            np.float64
        )  # [128 colors, 160 slots]
        mins = np.full((128, 128), np.inf)  # [chunk, color]
        for s in range(NSLOT):
            c = slot_chunk[s]
            if c >= 0:
                mins[c] = np.minimum(mins[c], md[:, s])
        d2 = mins + xn64  # [chunk, color]
        total += np.sqrt(np.maximum(d2, 0.0)).sum()
    out = np.array(total / N, dtype=np.float32)
    if _trace:
        kernel._last_results = res
    return out


if __name__ == "__main__":
    rng = np.random.default_rng(0)
    oc = rng.random((N, 3), dtype=np.float32)
    tp = rng.random((NP, 3), dtype=np.float32)
    got = kernel(output_colors=oc, target_palette=tp)
    d = oc[:, None, :] - tp[None, :, :]
    want = np.sqrt((d * d).sum(-1)).min(1).mean(dtype=np.float64)
    print("got", got, "want", want, "rel", abs(got - want) / abs(want))
